# revision 10
# baseline (speedup 1.0000x reference)
"""BitNetV3Attention (B=2, S=2048, H=16, DH=128, D=2048) on 8 TRN2 NeuronCores.

Strategy (tensor-parallel over heads + row-parallel o_proj):
  - Each core owns 2 of 16 heads. It computes Q^T/K^T (head-transposed,
    [DH, B*S]) and V ([B*S, DH]) for its heads from the full hidden states
    (replicated read), runs causal flash-style attention per (head, batch),
    producing normalized attn_out^T slices [256, B*S].
  - Two AllToAll collectives (one per local head slot) redistribute attn_out
    from head-sharded to sequence-sharded: core j ends with
    attn_out^T[:, rows_j] for ALL 2048 model dims, where rows_j are 512 rows
    of the [4096, 2048] token matrix. The first A2A fires after local head 0
    finishes and overlaps head 1's attention.
  - o_proj runs in two passes: pass 1 accumulates the even d-tiles (fed by
    A2A#0) into SBUF partials while A2A#1 is still in flight; pass 2
    accumulates the odd d-tiles and adds the partials. This keeps the PE
    busy across the second collective instead of stalling on it.
  - QKV startup: weight and hidden-state DMAs for the first s-chunk are
    split into k-pair chunks and interleaved so the first matmuls start
    ~4us in (DMA-paced) instead of waiting for the full 8MB preload.

All matmuls run in float32r (full-rate fp32 path at free-dim >= 256, which
every matmul here satisfies after clamping the causal-skip offset to 256).
Softmax skips max-subtraction (scores are O(5), exp is safe in fp32); the
padding mask rides the ScalarE activation's per-partition bias; the causal
mask is a single [128, 1024] additive template sliced per diagonal tile.
"""
import sys
for _p in ('/opt/trn_rl_repo', '/root/.axon_site/_ro/trn_rl_repo'):
    if _p not in sys.path:
        sys.path.append(_p)

import numpy as np

import concourse.mybir as mybir
import concourse.tile as tile
from concourse import bacc, bass_utils

B, S, H, DH = 2, 2048, 16, 128
D = H * DH                  # 2048
NS = B * S                  # 4096
NC = 8                      # cores
HL = H // NC                # 2 local heads
DSL = HL * DH               # 256 (d-slice per core)
ROWS = NS // NC             # 512 output rows per core
SCALE = 1.0 / float(np.sqrt(DH))
F32 = mybir.dt.float32
F32R = mybir.dt.float32r
BF16 = mybir.dt.bfloat16
MM_DT = F32R
EXP = mybir.ActivationFunctionType.Exp
NEG = -1.0e30

N_K = D // 128              # 16 contraction tiles
N_SC = NS // 512            # 8 s-chunks for QKV
N_QC = S // 512             # 4 q-chunks per batch


def build_bass(repeat=1, do_attn=True, do_a2a=True, do_oproj=True):
    nc = bacc.Bacc("TRN2", target_bir_lowering=False, debug=False, num_devices=NC)

    ht = nc.dram_tensor("ht", [D, NS], MM_DT, kind="ExternalInput").ap()
    wqt = nc.dram_tensor("wqt", [D, DSL], MM_DT, kind="ExternalInput").ap()
    wkt = nc.dram_tensor("wkt", [D, DSL], MM_DT, kind="ExternalInput").ap()
    wvt = nc.dram_tensor("wvt", [D, DSL], MM_DT, kind="ExternalInput").ap()
    wot = nc.dram_tensor("wot", [D, D], BF16, kind="ExternalInput").ap()
    pad = nc.dram_tensor("pad", [B, S], F32, kind="ExternalInput").ap()
    tri = nc.dram_tensor("tri", [128, 1024], F32, kind="ExternalInput").ap()
    onesd = nc.dram_tensor("ones", [128, 128], MM_DT, kind="ExternalInput").ap()
    out = nc.dram_tensor("out", [ROWS, D], F32, kind="ExternalOutput").ap()

    with tile.TileContext(nc) as tc:
        with tc.tile_pool(name="dram", bufs=1, space="DRAM") as dram, \
             tc.tile_pool(name="const", bufs=1) as cpool:
            a2a_in = [dram.tile([NC, DH, 512], BF16, name=f"a2a_in{h}") for h in range(HL)]
            a2a_out = [dram.tile([NC, DH, 512], BF16, name=f"a2a_out{h}") for h in range(HL)]

            tri_sb = cpool.tile([128, 1024], F32)
            pad_sb = cpool.tile([128, B * 16], F32)
            ones_sb = cpool.tile([128, 128], MM_DT)
            nc.sync.dma_start(
                pad_sb[:].rearrange("p (b t) -> p b t", b=B),
                pad.rearrange("b (t p) -> p b t", p=128),
            )
            nc.sync.dma_start(ones_sb[:], onesd)

            for _rep in range(repeat):
                # tri (512KB) is only needed at attention; load it after the
                # first body's QKV weight DMAs so it doesn't delay startup
                load_tri = (lambda: nc.sync.dma_start(tri_sb[:], tri)) \
                    if _rep == 0 else None
                _emit_body(nc, tc, a2a_in, a2a_out, tri_sb, pad_sb, ones_sb,
                           ht, wqt, wkt, wvt, wot, out, load_tri=load_tri,
                           do_attn=do_attn, do_a2a=do_a2a, do_oproj=do_oproj)
    nc.compile()
    return nc


def _emit_qkv(nc, tc, qt_sb, kt_sb, v_sb, ht, wqt, wkt, wvt, load_tri=None):
    with tc.tile_pool(name="wts", bufs=1) as wpool, \
         tc.tile_pool(name="hts", bufs=2) as hpool, \
         tc.tile_pool(name="ps1", bufs=1, space="PSUM") as pp1:
        srcs = {"q": wqt, "k": wkt, "v": wvt}
        w_sb = {nm: wpool.tile([128, N_K * DSL], MM_DT, name=f"w{nm}")
                for nm in srcs}
        ht_r = ht.rearrange("(k p) s -> p k s", p=128)

        for sc in range(N_SC):
            psq = [pp1.tile([128, 512], F32, tag=f"pq{h}", name=f"pq{h}") for h in range(HL)]
            psk = [pp1.tile([128, 512], F32, tag=f"pk{h}", name=f"pk{h}") for h in range(HL)]
            # V in natural [token, dh] layout: 4 m-tiles of [128 tok, 256 dh]
            psv = [pp1.tile([128, 2 * DH], F32, tag=f"pv{m}", name=f"pv{m}")
                   for m in range(4)]
            slabs = []
            if sc == 0:
                # interleave weight k-pairs with matching ht chunks so the
                # first matmuls start as soon as ~1.3MB has landed
                slabs = [hpool.tile([128, 8 * 512], MM_DT, tag="ht", name="htslab")
                         for _ in range(2)]
                # j=0 pair split at single-k granularity so the very first
                # matmuls start after only ~0.7MB of DMA
                for kk in range(2):
                    for nm in srcs:
                        nc.sync.dma_start(
                            w_sb[nm][:, DSL*kk:DSL*kk+DSL],
                            srcs[nm][128*kk:128*kk+128, :])
                    nc.sync.dma_start(
                        slabs[0][:, 512*kk:512*kk+512], ht_r[:, kk, 0:512])
                for j in range(1, 8):    # k-pair j = k tiles 2j, 2j+1
                    for nm in srcs:
                        nc.sync.dma_start(
                            w_sb[nm][:, DSL*2*j:DSL*2*j+2*DSL].rearrange(
                                "p (t m) -> p t m", t=2),
                            srcs[nm][256*j:256*j+256, :].rearrange(
                                "(t p) m -> p t m", p=128))
                    sl = slabs[j // 4]
                    c0 = 1024 * (j % 4)
                    nc.sync.dma_start(
                        sl[:, c0:c0+1024].rearrange("p (k s) -> p k s", k=2),
                        ht_r[:, 2*j:2*j+2, 0:512])
                if load_tri is not None:
                    load_tri()
            else:
                for half in range(2):
                    slab = hpool.tile([128, 8 * 512], MM_DT, tag="ht", name="htslab")
                    nc.sync.dma_start(
                        slab[:].rearrange("p (k s) -> p k s", k=8),
                        ht_r[:, 8*half:8*half+8, 512*sc:512*sc+512])
                    slabs.append(slab)
            for k in range(N_K):
                htt = slabs[k // 8][:, 512*(k % 8):512*(k % 8)+512]
                fl = dict(start=(k == 0), stop=(k == N_K - 1))
                for h in range(HL):
                    nc.tensor.matmul(
                        psq[h][:], w_sb["q"][:, DSL*k+128*h:DSL*k+128*h+128],
                        htt, **fl)
                    nc.tensor.matmul(
                        psk[h][:], w_sb["k"][:, DSL*k+128*h:DSL*k+128*h+128],
                        htt, **fl)
                # V directly in [token, dh] layout: stationary = ht tile,
                # streaming = wv columns for both heads (N=256, full rate)
                for m in range(4):
                    nc.tensor.matmul(
                        psv[m][:], htt[:, 128*m:128*m+128],
                        w_sb["v"][:, DSL*k:DSL*k+DSL], **fl)
            # drain PSUM -> SBUF, split across DVE and ACT
            for h in range(HL):
                nc.vector.tensor_copy(
                    qt_sb[h][:, 512*sc:512*sc+512], psq[h][:])
                nc.scalar.copy(
                    kt_sb[h][:, 512*sc:512*sc+512], psk[h][:])
            for m in range(4):
                st = 4 * sc + m
                for h in range(HL):
                    dst = v_sb[h][:, 128*st:128*st+128]
                    if (h + m) % 2 == 0:
                        nc.vector.tensor_copy(dst, psv[m][:, 128*h:128*h+128])
                    else:
                        nc.scalar.copy(dst, psv[m][:, 128*h:128*h+128])


def _emit_attention(nc, tc, qt_sb, kt_sb, v_sb, tri_sb, pad_sb, ones_sb,
                    a2a_in, a2a_out, do_a2a):
    with tc.tile_pool(name="att", bufs=1) as apool, \
         tc.tile_pool(name="ps2", bufs=1, space="PSUM") as pp2:
        for h in range(HL):
            for b in range(B):
                for qc in range(N_QC):
                    q0 = 512 * qc
                    n_sk = 4 * qc + 4
                    po = pp2.tile([128, 512], F32, tag="po", bufs=2, name="po")
                    pd = pp2.tile([128, 512], F32, tag="pd", bufs=2, name="pd")
                    for t in range(n_sk):
                        # columns sq < ot are fully causal-masked; skip them,
                        # but keep free-dim >= 256 (fp32r runs 1/4 rate below)
                        ot = max(0, 128 * t - q0)
                        o = min(ot, 256)
                        ps = pp2.tile([128, 512], F32, tag="ps", bufs=4, name="ps")
                        nc.tensor.matmul(
                            ps[:, o:512],
                            kt_sb[h][:, S*b+128*t:S*b+128*t+128],
                            qt_sb[h][:, S*b+q0+o:S*b+q0+512],
                            start=True, stop=True)
                        if t >= 4 * qc:  # diagonal block
                            nc.vector.tensor_add(
                                ps[:, o:512], ps[:, o:512],
                                tri_sb[:, 512-(ot-o):1024-ot])
                        ex = apool.tile([128, 512], MM_DT, tag="ex", bufs=6, name="ex")
                        nc.scalar.activation(
                            ex[:, o:512], ps[:, o:512], EXP,
                            bias=pad_sb[:, 16*b+t:16*b+t+1], scale=SCALE)
                        fl = dict(start=(t == 0), stop=(t == n_sk - 1))
                        st = 16 * b + t
                        nc.tensor.matmul(
                            po[:, o:512], v_sb[h][:, 128*st:128*st+128],
                            ex[:, o:512], **fl)
                        nc.tensor.matmul(
                            pd[:, o:512], ones_sb[:], ex[:, o:512], **fl)
                    rec = apool.tile([128, 512], F32, tag="rec", bufs=2, name="rec")
                    nc.vector.reciprocal(rec[:], pd[:])
                    ao = apool.tile([128, 512], BF16, tag="ao", bufs=2, name="ao")
                    nc.vector.tensor_mul(ao[:], po[:], rec[:])
                    nc.sync.dma_start(a2a_in[h][4*b+qc, :, :], ao[:])
            # ---- AllToAll for this head-slot (overlaps next head's attn) ----
            if do_a2a:
                nc.gpsimd.collective_compute(
                    "AllToAll", mybir.AluOpType.bypass,
                    replica_groups=[list(range(NC))],
                    ins=[a2a_in[h].opt()], outs=[a2a_out[h].opt()])


def _emit_oproj(nc, tc, opool, wopool, obpool, a2a_out, wot, out):
    # Two passes: evens (head slot 0, A2A#0) accumulate to SBUF partials
    # while A2A#1 flies; odds (A2A#1) accumulate in PSUM, then DVE adds the
    # partial and the result DMAs out.
    with tc.tile_pool(name="ps4", bufs=1, space="PSUM") as pp4:
        at_sb = [opool.tile([128, 8 * 512], BF16, name=f"at{half}")
                 for half in range(2)]
        for half in range(2):
            nc.sync.dma_start(
                at_sb[half][:].rearrange("p (j s) -> p j s", j=8),
                a2a_out[half].rearrange("j p s -> p j s"))
        part_sb = opool.tile([128, 16 * 512], F32, name="partial")
        # wot rows (t p) with t = global d-tile; split parity for slabs
        wot_r2 = wot.rearrange("(t2 two p) e -> p two t2 e", p=128, two=2)
        for half in range(2):
            for ne in range(4):
                sl = wopool.tile([128, 8 * 512], BF16, tag=f"wo{half}",
                                 name=f"wo{half}", bufs=2)
                nc.sync.dma_start(
                    sl[:].rearrange("p (t e) -> p t e", t=8),
                    wot_r2[:, half, :, 512*ne:512*ne+512])
                for m in range(4):
                    pout = pp4.tile([128, 512], F32, tag="pout", name="pout",
                                    bufs=4)
                    for i in range(8):   # d-tile g = 2i + half
                        nc.tensor.matmul(
                            pout[:],
                            at_sb[half][:, 512*i+128*m:512*i+128*m+128],
                            sl[:, 512*i:512*i+512],
                            start=(i == 0), stop=(i == 7))
                    idx = 4 * ne + m
                    if half == 0:
                        if idx % 2 == 0:
                            nc.vector.tensor_copy(
                                part_sb[:, 512*idx:512*idx+512], pout[:])
                        else:
                            nc.scalar.copy(
                                part_sb[:, 512*idx:512*idx+512], pout[:])
                    else:
                        ob = obpool.tile([128, 512], F32, tag="ob", name="ob",
                                         bufs=4)
                        nc.vector.tensor_add(
                            ob[:], pout[:], part_sb[:, 512*idx:512*idx+512])
                        nc.sync.dma_start(
                            out[128*m:128*m+128, 512*ne:512*ne+512], ob[:])


def _emit_body(nc, tc, a2a_in, a2a_out, tri_sb, pad_sb, ones_sb,
               ht, wqt, wkt, wvt, wot, out, load_tri=None,
               do_attn=True, do_a2a=True, do_oproj=True):
    with tc.tile_pool(name="store", bufs=1) as spool:
        qt_sb = [spool.tile([128, NS], MM_DT, name=f"qt{h}") for h in range(HL)]
        kt_sb = [spool.tile([128, NS], MM_DT, name=f"kt{h}") for h in range(HL)]
        v_sb = [spool.tile([128, NS], MM_DT, name=f"v{h}") for h in range(HL)]

        _emit_qkv(nc, tc, qt_sb, kt_sb, v_sb, ht, wqt, wkt, wvt,
                  load_tri=load_tri)

        # o_proj pools open before attention so Wo slab DMAs can prefetch
        # into the space vacated by the QKV weight/ht pools during attention.
        with tc.tile_pool(name="oproj", bufs=1) as opool, \
             tc.tile_pool(name="wo", bufs=2) as wopool, \
             tc.tile_pool(name="ob", bufs=3) as obpool:
            if do_attn:
                _emit_attention(nc, tc, qt_sb, kt_sb, v_sb, tri_sb, pad_sb,
                                ones_sb, a2a_in, a2a_out, do_a2a)
            if do_oproj:
                _emit_oproj(nc, tc, opool, wopool, obpool, a2a_out, wot, out)


_NC_CACHE = None


def _get_nc():
    global _NC_CACHE
    if _NC_CACHE is None:
        _NC_CACHE = build_bass()
    return _NC_CACHE


def make_in_maps(hidden_states, attention_mask, Wq, Wk, Wv, Wo):
    import ml_dtypes
    mm_np = np.float32 if MM_DT == F32R else ml_dtypes.bfloat16
    x = np.ascontiguousarray(np.asarray(hidden_states, dtype=np.float32)).reshape(NS, D)
    ht = np.ascontiguousarray(x.T).astype(mm_np)                     # [D, NS]
    wqt = np.ascontiguousarray(np.asarray(Wq, dtype=np.float32).T).astype(mm_np)
    wkt = np.ascontiguousarray(np.asarray(Wk, dtype=np.float32).T).astype(mm_np)
    wvt = np.ascontiguousarray(np.asarray(Wv, dtype=np.float32).T).astype(mm_np)
    wot = np.ascontiguousarray(
        np.asarray(Wo, dtype=np.float32).T).astype(ml_dtypes.bfloat16)
    mask = np.asarray(attention_mask)
    pad = np.where(mask == 0, np.float32(NEG), np.float32(0.0)).astype(np.float32)
    tri = np.where(
        np.arange(1024, dtype=np.int64)[None, :] >= np.arange(128, dtype=np.int64)[:, None] + 512,
        np.float32(0.0), np.float32(NEG)).astype(np.float32)
    ones = np.ones((128, 128), dtype=np.float32)

    in_maps = []
    for c in range(NC):
        sl = slice(DSL * c, DSL * c + DSL)
        in_maps.append({
            "ht": ht,
            "wqt": np.ascontiguousarray(wqt[:, sl]),
            "wkt": np.ascontiguousarray(wkt[:, sl]),
            "wvt": np.ascontiguousarray(wvt[:, sl]),
            "wot": wot,
            "pad": pad,
            "tri": tri,
            "ones": ones.astype(mm_np),
        })
    return in_maps


def assemble_output(results):
    rows = np.concatenate([results[c]["out"] for c in range(NC)], axis=0)
    return rows.reshape(B, S, D).astype(np.float32)


def kernel(hidden_states, attention_mask, Wq, Wk, Wv, Wo):
    nc = _get_nc()
    in_maps = make_in_maps(hidden_states, attention_mask, Wq, Wk, Wv, Wo)
    res = bass_utils.run_bass_kernel_spmd(nc, in_maps, core_ids=list(range(NC)))
    return assemble_output(res.results)
